# revision 18
# baseline (speedup 1.0000x reference)
"""PixelAttention Trainium2 kernel.

Computes, for each batch image (data-parallel, one image per NeuronCore):
    seq  = image.reshape(C, T).T            # [T, C], T = 32*32
    kqv  = seq @ w_kqv + b_kqv
    per-head causal attention (8 heads, head_dim 32), softmax over keys
    out  = mix(attn) + b_mix + image

Layout strategy (per core):
  - x kept as [C, T] (channels on partitions) -- image memory order directly.
  - k^T, q^T computed as [j, t]; QK^T contractions (K=32) run 4 heads
    row-packed in the PE array; logits land transposed L[s, t] so softmax
    normalization is a matmul-with-ones fold rather than a free-dim reduce.
  - exp on ScalarE (psum -> sbuf bf16); causal masking of diagonal blocks is
    a DVE/GpSimd multiply by an upper-triangular 0/1 tile (keeps PE streams
    geometry-uniform -- interleaving K=1/K=128 matmuls into the K=32 stream
    destroys PE pipelining and keeps the HAM clock gate cold).
  - out_un^T = [V | 1]^T E accumulated over s-tiles (M=33: psum row 32 is the
    softmax denominator), 2 heads col-packed per matmul.
  - division by denominator via DVE reciprocal + selector-matmul broadcast;
    the division tail and mix run after both chunks so PSUM slot rotation
    never blocks the attention stream.
  - mix as w_mix^T @ attn^T; biases and residual folded into DVE ops.

All matmul operands are bf16 (fp32 PSUM accumulation); residual path fp32.
"""

import numpy as np
import ml_dtypes

import concourse.bass as bass
import concourse.tile as tile
from concourse import bacc, mybir
from concourse.bass_utils import run_bass_kernel_spmd

BF = ml_dtypes.bfloat16
T, C, H, D = 1024, 256, 8, 32
N_CORES = 8

_CACHE = {}


def _build_nc():
    f32 = mybir.dt.float32
    bf16 = mybir.dt.bfloat16
    EXP = mybir.ActivationFunctionType.Exp
    ADD = mybir.AluOpType.add

    nc = bacc.Bacc("TRN2", target_bir_lowering=False, debug=False)

    def din(name, shape, dt):
        return nc.dram_tensor(name, shape, dt, kind="ExternalInput").ap()

    x_bf = din("x_bf", [C, T], bf16)
    x_f = din("x_f", [C, T], f32)
    wk = din("wk", [C, 256], bf16)
    wq = din("wq", [C, 256], bf16)
    wv = din("wv", [C, 256], bf16)
    wm = din("wm", [C, 256], bf16)
    bjt = din("bjt", [4, 128], f32)   # per-partition bias for qkT tiles
    bv = din("bv", [1, 256], bf16)    # broadcast along partitions
    bm2 = din("bm2", [2, 128], f32)   # per-partition bias for mix output
    tri = din("tri", [128, 128], bf16)  # tri[p, q] = 1 if q >= p else 0
    sel = din("sel", [8, 256], bf16)
    y = nc.dram_tensor("y", [C, T], f32, kind="ExternalOutput").ap()

    with tile.TileContext(nc) as tc:
        with (
            tc.tile_pool(name="consts", bufs=1) as consts,
            tc.tile_pool(name="sb", bufs=4) as sb,
            tc.tile_pool(name="ps", bufs=2, space="PSUM") as ps_pool,
            tc.tile_pool(name="lpp", bufs=3, space="PSUM") as lp_pool,
            tc.tile_pool(name="dr", bufs=2, space="DRAM") as dram_pool,
        ):
            # ---- constant loads (critical path first; bulk on other queues) ----
            xb = consts.tile([128, 2, T], bf16)
            x_bf_r = x_bf.rearrange("(a p) t -> p a t", p=128)
            nc.sync.dma_start(out=xb[:, 0, :], in_=x_bf_r[:, 0, :])
            nc.sync.dma_start(out=xb[:, 1, :], in_=x_bf_r[:, 1, :])
            w_tiles = {}
            for eng, (name, ap) in zip(
                (nc.sync, nc.sync, nc.scalar, nc.scalar),
                (("wk", wk), ("wq", wq), ("wv", wv), ("wm", wm)),
            ):
                t_ = consts.tile([128, 2, 256], bf16, tag=name, name=name)
                eng.dma_start(out=t_, in_=ap.rearrange("(a p) j -> p a j", p=128))
                w_tiles[name] = t_
            bjt_sb = consts.tile([128, 4], f32)
            nc.scalar.dma_start(out=bjt_sb, in_=bjt.rearrange("a p -> p a"))
            bv_bc = consts.tile([128, 256], bf16)
            nc.scalar.dma_start(
                out=bv_bc,
                in_=bass.AP(tensor=bv.tensor, offset=bv.offset,
                            ap=[[0, 128]] + list(bv.ap[1:])),
            )
            tri_sb = consts.tile([128, 128], bf16)
            nc.gpsimd.dma_start(out=tri_sb, in_=tri)
            sel_s = consts.tile([8, 256], bf16)
            nc.gpsimd.dma_start(out=sel_s, in_=sel)
            xf = consts.tile([128, 2, T], f32)
            nc.gpsimd.dma_start(out=xf, in_=x_f.rearrange("(a p) t -> p a t", p=128))
            bm2_sb = consts.tile([128, 2], f32)
            nc.gpsimd.dma_start(out=bm2_sb, in_=bm2.rearrange("a p -> p a"))

            warm = consts.tile([128, 1], f32, tag="warm", name="warm")
            nc.scalar.activation(out=warm, in_=bjt_sb[:, 0:1], func=EXP)

            qkT = consts.tile([128, 4, T], bf16)  # jt 0-1: kT, 2-3: qT
            vsb = consts.tile([128, 8, H, 33], bf16)  # [p, s_tile, head, v|1]
            nc.vector.memset(vsb[:, :, :, 32:33], 1.0)
            attnT = consts.tile([128, 2, T], bf16)

            # ---- phase 1: kT/qT projections ([j, t] layout) ----
            plan = [
                (0, "wk", 0),
                (2, "wq", 0),
                (1, "wk", 1),
                (3, "wq", 1),
            ]
            for dst, wname, jl in plan:
                w_s = w_tiles[wname]
                js = slice(jl * 128, (jl + 1) * 128)
                for tch in range(2):
                    ts_ = slice(tch * 512, (tch + 1) * 512)
                    p = lp_pool.tile([128, 2, 512], f32, tag="lp",
                                     name="pp")[:, 0, :]
                    nc.tensor.matmul(
                        out=p, lhsT=w_s[:, 0, js], rhs=xb[:, 0, ts_],
                        start=True, stop=False,
                    )
                    nc.tensor.matmul(
                        out=p, lhsT=w_s[:, 1, js], rhs=xb[:, 1, ts_],
                        start=False, stop=True,
                    )
                    nc.vector.tensor_scalar(
                        out=qkT[:, dst, ts_], in0=p,
                        scalar1=bjt_sb[:, dst:dst + 1], scalar2=None, op0=ADD,
                    )

            # ---- phase 2: V ([t, j] layout, ones column appended) ----
            for st in range(8):
                ss = slice(st * 128, (st + 1) * 128)
                p = lp_pool.tile([128, 2, 512], f32, tag="lp",
                                 name="pv")[:, 0, 0:256]
                nc.tensor.matmul(
                    out=p, lhsT=xb[:, 0, ss], rhs=w_tiles["wv"][:, 0, :],
                    start=True, stop=False,
                )
                nc.tensor.matmul(
                    out=p, lhsT=xb[:, 1, ss], rhs=w_tiles["wv"][:, 1, :],
                    start=False, stop=True,
                )
                nc.vector.tensor_add(
                    out=vsb[:, st, :, 0:32],
                    in0=p.rearrange("p (h e) -> p h e", e=32),
                    in1=bv_bc.rearrange("p (h e) -> p h e", e=32),
                )

            # ---- phase 3: attention ----
            pending = []
            for c in range(2):
                ouc = {}
                rs = sb.tile([8, 512], f32, tag="rs", name="rs", bufs=2)
                for g in range(2):
                    po = {
                        0: ps_pool.tile([128, 512], f32, tag="po", name="po0"),
                        1: ps_pool.tile([128, 512], f32, tag="po", name="po1"),
                    }
                    for st in range(4 * c + 4):
                        tlo = 128 * st - 512 * c
                        diag = tlo >= 0
                        if not diag:
                            tlo = 0
                        for pair in range(2):
                            lp = lp_pool.tile([128, 2, 512], f32, tag="lp", name="lp")
                            E = sb.tile([128, 2, 512], bf16, tag="E", name="E", bufs=8)
                            for h2 in range(2):
                                hl = 2 * pair + h2
                                rp = 32 * hl
                                kT_l = qkT[rp:rp + 32, g, st * 128:(st + 1) * 128]
                                qg = qkT[rp:rp + 32, 2 + g, :]
                                nc.tensor.matmul(
                                    out=lp[:, h2, tlo:512], lhsT=kT_l,
                                    rhs=qg[:, c * 512 + tlo:(c + 1) * 512],
                                    start=True, stop=True,
                                    tile_position=(rp, 0),
                                )
                            nc.scalar.activation(
                                out=E[:, :, tlo:512], in_=lp[:, :, tlo:512],
                                func=EXP,
                            )
                            if diag:
                                tri_b = bass.AP(
                                    tensor=tri_sb.tensor, offset=tri_sb.offset,
                                    ap=[list(tri_sb.ap[0]), [0, 2]]
                                       + list(tri_sb.ap[1:]),
                                )
                                nc.gpsimd.tensor_mul(
                                    out=E[:, :, tlo:tlo + 128],
                                    in0=E[:, :, tlo:tlo + 128],
                                    in1=tri_b,
                                )
                            for h2 in range(2):
                                hl = 2 * pair + h2
                                h = 4 * g + hl
                                nc.tensor.matmul(
                                    out=po[pair][64 * h2:64 * h2 + 33, tlo:512],
                                    lhsT=vsb[:, st, h, :], rhs=E[:, h2, tlo:512],
                                    start=(st == 0), stop=(st == 4 * c + 3),
                                    skip_group_check=True,
                                    tile_position=(0, 64 * h2),
                                )
                    # division part 1 for this (c, g): psum -> sbuf, compact
                    t_ = sb.tile([128, 512], f32, tag="ouc", name="ouc", bufs=4)
                    ou = {}
                    for pair in range(2):
                        t_ou = sb.tile([128, 512], f32, tag="ou", name="ou")
                        nc.vector.tensor_copy(out=t_ou, in_=po[pair])
                        ou[pair] = t_ou
                    for pair in range(2):
                        for h2 in range(2):
                            hl = 2 * pair + h2
                            h = 4 * g + hl
                            nc.gpsimd.dma_start(
                                out=t_[32 * hl:32 * hl + 32, :],
                                in_=ou[pair][64 * h2:64 * h2 + 32, :],
                            )
                            nc.sync.dma_start(
                                out=rs[h:h + 1, :],
                                in_=ou[pair][64 * h2 + 32:64 * h2 + 33, :],
                            )
                    ouc[g] = t_
                rcf = sb.tile([8, 512], f32, tag="rcf", name="rcf", bufs=2)
                nc.vector.reciprocal_approx_fast(out=rcf, in_=rs)
                rcb = sb.tile([8, 512], bf16, tag="rcb", name="rcb", bufs=2)
                nc.vector.tensor_copy(out=rcb, in_=rcf)
                pending.append((c, ouc, rcb))

            # ---- division part 2 + mix + residual (off the PE critical path) ----
            for c, ouc, rcb in pending:
                cs = slice(c * 512, (c + 1) * 512)
                for g in range(2):
                    bc = ps_pool.tile([128, 512], f32, tag="po", name="bc")
                    nc.tensor.matmul(
                        out=bc, lhsT=sel_s[:, g * 128:(g + 1) * 128], rhs=rcb,
                        start=True, stop=True,
                    )
                    nc.vector.tensor_mul(out=attnT[:, g, cs], in0=ouc[g], in1=bc)
                for c2t in range(2):
                    c2s = slice(c2t * 128, (c2t + 1) * 128)
                    mp = lp_pool.tile([128, 2, 512], f32, tag="lp",
                                      name="mp")[:, 0, :]
                    nc.tensor.matmul(
                        out=mp, lhsT=w_tiles["wm"][:, 0, c2s], rhs=attnT[:, 0, cs],
                        start=True, stop=False,
                    )
                    nc.tensor.matmul(
                        out=mp, lhsT=w_tiles["wm"][:, 1, c2s], rhs=attnT[:, 1, cs],
                        start=False, stop=True,
                    )
                    os_ = sb.tile([128, 512], f32, tag="os", name="os")
                    nc.vector.scalar_tensor_tensor(
                        out=os_, in0=mp, scalar=bm2_sb[:, c2t:c2t + 1],
                        in1=xf[:, c2t, cs], op0=ADD, op1=ADD,
                    )
                    nc.sync.dma_start(
                        out=y.rearrange("(a p) t -> p a t", p=128)[:, c2t, cs],
                        in_=os_,
                    )

    nc.compile()
    return nc


def _host_inputs(image, w_kqv, b_kqv, w_mix, b_mix):
    s = np.float32(1.0 / np.sqrt(D))
    wk = w_kqv[:, :256]
    wq = w_kqv[:, 256:512] * s
    wv = w_kqv[:, 512:]
    bk = b_kqv[:256].astype(np.float32)
    bq = (b_kqv[256:512] * s).astype(np.float32)
    bv = b_kqv[512:].reshape(1, 256)
    bjt = np.stack([bk[0:128], bk[128:256], bq[0:128], bq[128:256]])
    bm2 = np.asarray(b_mix, np.float32).reshape(2, 128)
    tri = (np.arange(128)[None, :] >= np.arange(128)[:, None]).astype(np.float32)
    sel = np.zeros((8, 256), np.float32)
    for h in range(8):
        g, hl = divmod(h, 4)
        sel[h, 128 * g + 32 * hl:128 * g + 32 * hl + 32] = 1.0
    common = {
        "wk": np.ascontiguousarray(wk).astype(BF),
        "wq": np.ascontiguousarray(wq).astype(BF),
        "wv": np.ascontiguousarray(wv).astype(BF),
        "wm": np.ascontiguousarray(w_mix).astype(BF),
        "bjt": np.ascontiguousarray(bjt),
        "bv": bv.astype(BF),
        "bm2": bm2,
        "tri": tri.astype(BF),
        "sel": sel.astype(BF),
    }
    in_maps = []
    for i in range(N_CORES):
        x = np.ascontiguousarray(image[i].reshape(C, T)).astype(np.float32)
        in_maps.append({**common, "x_f": x, "x_bf": x.astype(BF)})
    return in_maps


def _run(inputs, trace=False):
    if "nc" not in _CACHE:
        _CACHE["nc"] = _build_nc()
    nc = _CACHE["nc"]
    in_maps = _host_inputs(
        np.asarray(inputs["image"], np.float32),
        np.asarray(inputs["w_kqv"], np.float32),
        np.asarray(inputs["b_kqv"], np.float32),
        np.asarray(inputs["w_mix"], np.float32),
        np.asarray(inputs["b_mix"], np.float32),
    )
    res = run_bass_kernel_spmd(nc, in_maps, list(range(N_CORES)), trace=trace)
    out = np.stack(
        [np.asarray(res.results[i]["y"]).reshape(C, 32, 32) for i in range(N_CORES)]
    ).astype(np.float32)
    return out, res


def kernel(**inputs):
    out, _ = _run(inputs, trace=False)
    return out
